# revision 1
# baseline (speedup 1.0000x reference)
"""Expert-parallel MoE layer for Trainium2 (Bass/Tile, 8 NeuronCores).

Strategy (sharding hardcoded for B=4, T=2048, C=1024, E=8, H=2728, top_k=2):
  - Expert-parallel: core e owns expert e's weights (w1/w2/w3[e]).
  - Host computes the router *selection* only (top-2 expert ids per token;
    verified identical across numpy/jax-cpu/jax-neuron fp32 paths for this
    regime) and performs the all-to-all token dispatch/combine as the
    shard/unshard step.
  - Each core, on device: recomputes gate logits for its tokens (gate_w is
    replicated), derives its softmax combine weight g = exp(l_e - m1) /
    (exp(m1 - m1) + exp(m2 - m1)) per token, computes the full expert FFN
    y = (silu(x@w1.T) * (x@w3.T)) @ w2.T, scales by g, and returns the
    per-expert partial outputs, which the host scatter-adds.

Matmuls run in float32r (fp32 storage, full PE rate, ~11-bit mantissa);
inputs are pre-rounded on the host so DRAM tensors can be declared float32r
and DMA'd straight into PE-ready SBUF tiles. Layouts are pre-arranged on the
host (partition-major) so big transfers are single contiguous DMAs, spread
across several engine queues.
"""

import os
import sys
from contextlib import ExitStack

import numpy as np

for _p in ("/opt/trn_rl_repo", "/root/.axon_site/_ro/trn_rl_repo"):
    if os.path.isdir(_p) and _p not in sys.path:
        sys.path.insert(0, _p)

import concourse.mybir as mybir
import concourse.tile as tile
from concourse.tile_rust import add_dep_helper
from concourse import bacc
from concourse.bass_utils import run_bass_kernel_spmd

FP32 = mybir.dt.float32
F32R = mybir.dt.float32r  # fp32 storage, PE matmul at full rate (~11-bit mantissa)
ALU = mybir.AluOpType
AF = mybir.ActivationFunctionType
AX = mybir.AxisListType

E = 8            # experts == cores
C = 1024         # model dim
H0 = 2728        # ffn hidden dim
KC = C // 128    # 8 contraction tiles over C
KH = (H0 + 127) // 128  # 22 tiles over padded H
HP = KH * 128    # 2816
TT = 512         # max token tile (fp32 PSUM bank = 512 floats)
# s reload split: 4 groups of h-tiles so phase B's first matmuls only wait
# on a quarter of each token tile's s block
S_GRP = [(0, 2), (2, 6), (8, 7), (15, 7)]
CAP_MAX = 2304   # per-launch token cap (SBUF budget); split into runs beyond

_CACHE = {}
LAST_RESULTS = None


def _token_tiles(cap):
    # all tiles >= 256 wide: float32r matmuls below 256 columns run at a
    # 4x/2x cycles-per-row penalty, so a narrow tail tile costs as much as
    # a full 512 tile. Sub-512 tiles go FIRST: the first matmul's DMA
    # dependency is smaller, so the PE starts (and ramps) earlier.
    widths = []
    left = cap
    while left > 640:
        widths.append(TT)
        left -= TT
    if left == 640:
        widths = [384, 256] + widths
    elif left > 0:
        widths = [left] + widths
    widths.sort()
    tiles = []
    off = 0
    for w in widths:
        tiles.append((off, w))
        off += w
    return tiles


def _preround(a):
    """Round fp32 array to float32r precision (round-to-nearest with the 12
    low mantissa bits dropped) so DRAM tensors can be declared float32r and
    DMA'd straight into PE-ready SBUF tiles with no on-device conversion."""
    v = np.ascontiguousarray(a, dtype=np.float32).view(np.uint32)
    r = ((v.astype(np.uint64) + 0x800) & 0xFFFFF000).astype(np.uint32)
    return r.view(np.float32)


def _build(cap):
    """Build + compile the SPMD program for `cap` tokens per core."""
    assert cap % 128 == 0
    NTT = cap // 128        # 128-token tiles (gate)
    tiles = _token_tiles(cap)
    nc = bacc.Bacc("TRN2", target_bir_lowering=False, debug=False, num_devices=E)

    xs = nc.dram_tensor("xs", [KC, 128, cap], F32R, kind="ExternalInput").ap()
    w1s = nc.dram_tensor("w1s", [KH, 128, C], F32R, kind="ExternalInput").ap()
    w3s = nc.dram_tensor("w3s", [KH, 128, C], F32R, kind="ExternalInput").ap()
    w2s = nc.dram_tensor("w2s", [KC, 128, KH * 128], F32R, kind="ExternalInput").ap()
    gws = nc.dram_tensor("gws", [128, KC, E], F32R, kind="ExternalInput").ap()
    esl = nc.dram_tensor("esl", [128, E], FP32, kind="ExternalInput").ap()
    yt = nc.dram_tensor("yt", [KC, 128, cap], FP32, kind="ExternalOutput").ap()

    with tile.TileContext(nc) as tc, ExitStack() as top:
        dramp = top.enter_context(tc.tile_pool(name="dram", bufs=1, space="DRAM"))
        constp = top.enter_context(tc.tile_pool(name="const", bufs=1))
        gresp = top.enter_context(tc.tile_pool(name="gres", bufs=1))

        s_dram = dramp.tile([128, KH, cap], F32R)
        g_dram = dramp.tile([cap], FP32)

        # gate consts ride the ACT queue: off the sync head (which must
        # deliver w[h0]+x[t0] ASAP) but still landed long before the gate
        gw_sb = constp.tile([128, KC, E], F32R)
        nc.scalar.dma_start(gw_sb[:], gws[:])
        es_sb = constp.tile([128, E], FP32)
        nc.scalar.dma_start(es_sb[:], esl[:])

        gcol = gresp.tile([128, NTT], FP32)

        # w2 is resident for the whole kernel; its loads are interleaved
        # into phase A's h-loop (below) so they hide behind compute without
        # starving the startup x/w1/w3 streams of DMA bandwidth
        w2p = top.enter_context(tc.tile_pool(name="w2res", bufs=1))
        w2_sb = [w2p.tile([128, KH, 128], F32R, tag=f"w2_{c}",
                          name=f"w2_sb_{c}") for c in range(KC)]

        anchor = None
        with ExitStack() as pha:
            xp = pha.enter_context(tc.tile_pool(name="xres", bufs=1))
            x_sb = [xp.tile([128, cap], F32R, tag=f"x{k}", name=f"x_sb{k}")
                    for k in range(KC)]
            # startup-critical loads share the SYNC queue in exact
            # consumption order (per-queue DMA processing is FIFO). They
            # must NOT ride the ACT queue: ACT's in-order sequencer would
            # sit in dma-issue instructions instead of running phase A's
            # silu ops, stalling PSUM slot recycling and starving the PE.
            wst = pha.enter_context(tc.tile_pool(name="wst", bufs=4))
            w_pre = {}
            for h in range(min(4, KH)):
                w1p_ = wst.tile([128, C], F32R, tag="w1", name=f"w1_sb{h}")
                w3p_ = wst.tile([128, C], F32R, tag="w3", name=f"w3_sb{h}")
                w_pre[h] = (w1p_, w3p_)
            nc.sync.dma_start(w_pre[0][0][:], w1s[0])
            nc.sync.dma_start(w_pre[0][1][:], w3s[0])
            w_loaded = {0}
            for ti, (to, tw) in enumerate(tiles):
                for k in range(KC):
                    nc.sync.dma_start(x_sb[k][:, to:to + tw],
                                      xs[k][:, to:to + tw])
                # weight rows for h=1..3 slot into the x stream in
                # consumption order (per-queue processing is FIFO, and the
                # tiles are pre-allocated so nothing delays the issue)
                hh = ti + 1
                if ti <= 2 and hh in w_pre and hh not in w_loaded:
                    nc.sync.dma_start(w_pre[hh][0][:], w1s[hh])
                    nc.sync.dma_start(w_pre[hh][1][:], w3s[hh])
                    w_loaded.add(hh)
            for hh in sorted(set(w_pre) - w_loaded - {0}):
                nc.sync.dma_start(w_pre[hh][0][:], w1s[hh])
                nc.sync.dma_start(w_pre[hh][1][:], w3s[hh])

            # ---- gate: logits -> per-token combine weight for this expert.
            # Emitted (below) after the startup h-batch: its ACT exp ops for
            # the last token tiles would otherwise precede phase A's silus
            # in ACT's in-order stream while waiting on the last x chunks.
            psg = pha.enter_context(tc.tile_pool(name="psg", bufs=2, space="PSUM"))
            gtmp = pha.enter_context(tc.tile_pool(name="gtmp", bufs=2))

            def emit_gate():
              for tt in range(NTT):
                  sl = slice(tt * 128, (tt + 1) * 128)
                  pl = psg.tile([128, E], FP32, tag="pl", name=f"pl{tt}")
                  for k in range(KC):
                      nc.tensor.matmul(pl[:], x_sb[k][:, sl], gw_sb[:, k, :],
                                       start=(k == 0), stop=(k == KC - 1))
                  l_sb = gtmp.tile([128, E], FP32, tag="l", name=f"l{tt}")
                  nc.vector.tensor_copy(l_sb[:], pl[:])
                  m1 = gtmp.tile([128, 1], FP32, tag="m1", name=f"m1_{tt}")
                  nc.vector.reduce_max(m1[:], l_sb[:], axis=AX.X)
                  eq = gtmp.tile([128, E], FP32, tag="eq", name=f"eq{tt}")
                  nc.vector.tensor_scalar(eq[:], l_sb[:], m1[:], None, ALU.is_equal)
                  eqb = gtmp.tile([128, E], FP32, tag="eqb", name=f"eqb{tt}")
                  nc.vector.tensor_scalar(eqb[:], eq[:], 1e30, None, ALU.mult)
                  msk = gtmp.tile([128, E], FP32, tag="msk", name=f"msk{tt}")
                  nc.vector.tensor_tensor(msk[:], l_sb[:], eqb[:], op=ALU.subtract)
                  m2 = gtmp.tile([128, 1], FP32, tag="m2", name=f"m2_{tt}")
                  nc.vector.reduce_max(m2[:], msk[:], axis=AX.X)
                  sel = gtmp.tile([128, E], FP32, tag="sel", name=f"sel{tt}")
                  nc.vector.tensor_tensor(sel[:], l_sb[:], es_sb[:], op=ALU.mult)
                  le = gtmp.tile([128, 1], FP32, tag="le", name=f"le{tt}")
                  nc.vector.reduce_sum(le[:], sel[:], axis=AX.X)
                  nm1 = gtmp.tile([128, 1], FP32, tag="nm1", name=f"nm1_{tt}")
                  nc.vector.tensor_scalar(nm1[:], m1[:], -1.0, None, ALU.mult)
                  ea = gtmp.tile([128, 1], FP32, tag="ea", name=f"ea{tt}")
                  nc.scalar.activation(ea[:], le[:], AF.Exp, bias=nm1[:])
                  eb = gtmp.tile([128, 1], FP32, tag="eb", name=f"eb{tt}")
                  nc.scalar.activation(eb[:], m2[:], AF.Exp, bias=nm1[:])
                  den = gtmp.tile([128, 1], FP32, tag="den", name=f"den{tt}")
                  nc.vector.tensor_scalar(den[:], eb[:], 1.0, None, ALU.add)
                  rec = gtmp.tile([128, 1], FP32, tag="rec", name=f"rec{tt}")
                  nc.vector.reciprocal(rec[:], den[:])
                  nc.vector.tensor_tensor(gcol[:, tt:tt + 1], ea[:], rec[:],
                                          op=ALU.mult)
              # g: [128-token partition] columns -> flat HBM (gpsimd queue:
              # only ready once the gate finishes; would head-block an
              # in-order HWDGE queue shared with the weight streams). The
              # row reload + partition broadcast happen at the top of B.
              nc.gpsimd.dma_start(
                  g_dram.rearrange("(t p) -> p t", p=128), gcol[:])

            # ---- phase A: s = silu(x@w1.T) * (x@w3.T), token-major in HBM.
            # Emission order: h0..h3 over all tiles but the last, then the
            # last tile for h0..h3, then h4+. Pool slots recycle in
            # allocation order, so putting the late-arriving last x tile's
            # work in a trailing batch keeps its stall out of the slot
            # chain that feeds h2/h3 during the x stream.
            psa = pha.enter_context(tc.tile_pool(name="psA", bufs=3, space="PSUM"))
            stg = pha.enter_context(tc.tile_pool(name="stg", bufs=3))
            anchors = {}

            def emit_ht(h, t, w1_sb, w3_sb):
                nonlocal anchor
                to, tw = tiles[t]
                p1 = psa.tile([128, TT], FP32, tag="p1", name=f"p1_{h}_{t}")
                p3 = psa.tile([128, TT], FP32, tag="p3", name=f"p3_{h}_{t}")
                for k in range(KC):
                    nc.tensor.matmul(p1[:, :tw],
                                     w1_sb[:, k * 128:(k + 1) * 128],
                                     x_sb[k][:, to:to + tw],
                                     start=(k == 0), stop=(k == KC - 1))
                for k in range(KC):
                    anchor = nc.tensor.matmul(
                        p3[:, :tw], w3_sb[:, k * 128:(k + 1) * 128],
                        x_sb[k][:, to:to + tw],
                        start=(k == 0), stop=(k == KC - 1))
                anchors[(h, t)] = anchor
                sa = stg.tile([128, TT], FP32, tag="sa", name=f"sa{h}_{t}")
                nc.scalar.activation(sa[:, :tw], p1[:, :tw], AF.Silu)
                so = stg.tile([128, TT], F32R, tag="so", name=f"so{h}_{t}")
                nc.vector.tensor_tensor(so[:, :tw], sa[:, :tw], p3[:, :tw],
                                        op=ALU.mult)
                nc.scalar.dma_start(s_dram[:, h, to:to + tw], so[:, :tw])

            last = len(tiles) - 1
            npre = min(4, KH)
            seq = [(h, t) for h in range(npre) for t in range(last)]
            seq += [(h, last) for h in range(npre)]
            w_cur = dict(w_pre)
            for h, t in seq:
                emit_ht(h, t, *w_cur[h])
            emit_gate()
            seq = [(h, t) for h in range(npre, KH) for t in range(len(tiles))]
            for h, t in seq:
                if h not in w_cur:
                    w1_sb = wst.tile([128, C], F32R, tag="w1", name=f"w1_sb{h}")
                    nc.sync.dma_start(w1_sb[:], w1s[h])
                    w3_sb = wst.tile([128, C], F32R, tag="w3", name=f"w3_sb{h}")
                    nc.sync.dma_start(w3_sb[:], w3s[h])
                    w_cur[h] = (w1_sb, w3_sb)
                    if npre <= h <= npre + 2 * KC - 2 and (h - npre) % 2 == 0:
                        c = (h - npre) // 2
                        w2dma = nc.gpsimd.dma_start(
                            w2_sb[c][:],
                            w2s[c].rearrange("p (h j) -> p h j", h=KH))
                        # hold each w2 load back until phase A is under way,
                        # so the startup x/w1/w3 streams keep the DMA
                        # engines to themselves
                        add_dep_helper(w2dma.ins, anchor.ins,
                                       reason="delay w2 prefetch")
                emit_ht(h, t, *w_cur[h])

        # ---- phase B: y = (s @ w2.T) * g ----
        with ExitStack() as phb:
            gbc = phb.enter_context(tc.tile_pool(name="gbc", bufs=1))
            g_sb = []
            for t, (to, tw) in enumerate(tiles):
                grow = gbc.tile([1, TT], FP32, tag="grow", name=f"grow{t}",
                                bufs=2)
                nc.gpsimd.dma_start(grow[0:1, :tw], g_dram[to:to + tw])
                gt = gbc.tile([128, tw], FP32, tag=f"g{t}", name=f"g_sb{t}")
                nc.gpsimd.partition_broadcast(gt[:], grow[0:1, :tw])
                g_sb.append(gt)
            sinp = phb.enter_context(tc.tile_pool(name="sin", bufs=2))
            psb = phb.enter_context(tc.tile_pool(name="psB", bufs=3, space="PSUM"))
            yp = phb.enter_context(tc.tile_pool(name="yst", bufs=4))
            # smallest tile first (quick phase entry) and second-smallest
            # last (short final drain); the middle in arbitrary order
            b_order = sorted(range(len(tiles)),
                             key=lambda i: (i != 0, -tiles[i][1]))
            for t in b_order:
                to, tw = tiles[t]
                s_t = []
                for q, (h0, hn) in enumerate(S_GRP):
                    sq = sinp.tile([128, hn, TT], F32R, tag=f"sq{q}",
                                   name=f"s_q{q}_{t}")
                    nc.scalar.dma_start(sq[:, :, :tw],
                                        s_dram[:, h0:h0 + hn, to:to + tw])
                    s_t.append(sq)
                for c in range(KC):
                    py = psb.tile([128, TT], FP32, tag="py", name=f"py{t}_{c}")
                    for q, (h0, hn) in enumerate(S_GRP):
                        for hh in range(hn):
                            h = h0 + hh
                            nc.tensor.matmul(py[:, :tw], w2_sb[c][:, h, :],
                                             s_t[q][:, hh, :tw],
                                             start=(h == 0),
                                             stop=(h == KH - 1))
                    yb = yp.tile([128, TT], FP32, tag="y", name=f"yb{t}_{c}")
                    nc.vector.tensor_tensor(yb[:, :tw], py[:, :tw], g_sb[t][:],
                                            op=ALU.mult)
                    nc.sync.dma_start(yt[c, :, to:to + tw], yb[:, :tw])

    nc.compile()
    return nc


def kernel(x, gate_w, w1, w2, w3, top_k):
    global LAST_RESULTS
    x = np.asarray(x, dtype=np.float32)
    gw = np.asarray(gate_w, dtype=np.float32)
    w1 = np.asarray(w1, dtype=np.float32)
    w2 = np.asarray(w2, dtype=np.float32)
    w3 = np.asarray(w3, dtype=np.float32)
    assert int(np.asarray(top_k)) == 2
    Bb, T, Cc = x.shape
    N = Bb * T
    assert Cc == C and w1.shape == (E, H0, C)

    xf = np.ascontiguousarray(x.reshape(N, C))
    # Router selection on host (dispatch is the sharding step); the gate
    # weights actually applied to the output are recomputed on device.
    logits = xf @ gw.T
    order = np.argsort(-logits, axis=1, kind="stable")[:, :2]
    tok = [np.nonzero((order == e).any(axis=1))[0] for e in range(E)]

    gws_np = _preround(
        np.ascontiguousarray(gw.T.reshape(KC, 128, E).transpose(1, 0, 2)))
    wmaps = []
    for e in range(E):
        w1t = np.zeros((C, HP), np.float32)
        w1t[:, :H0] = w1[e].T
        w1s_np = _preround(np.ascontiguousarray(
            w1t.reshape(KC, 128, KH, 128).transpose(2, 1, 0, 3)).reshape(KH, 128, C))
        w3t = np.zeros((C, HP), np.float32)
        w3t[:, :H0] = w3[e].T
        w3s_np = _preround(np.ascontiguousarray(
            w3t.reshape(KC, 128, KH, 128).transpose(2, 1, 0, 3)).reshape(KH, 128, C))
        w2t = np.zeros((HP, C), np.float32)
        w2t[:H0] = w2[e].T
        w2s_np = _preround(np.ascontiguousarray(
            w2t.reshape(KH, 128, KC, 128).transpose(2, 1, 0, 3)).reshape(KC, 128, KH * 128))
        es_np = np.zeros((128, E), np.float32)
        es_np[:, e] = 1.0
        wmaps.append({"w1s": w1s_np, "w3s": w3s_np, "w2s": w2s_np,
                      "gws": gws_np, "esl": es_np})

    out = np.zeros((N, C), np.float32)
    # normally one launch; if an expert ever holds > CAP_MAX tokens, split
    # tokens into several SPMD launches (FFN + gate weight are per-token)
    nchunk = (max(t.size for t in tok) + CAP_MAX - 1) // CAP_MAX
    for ci in range(nchunk):
        tokc = [t[(ci * t.size) // nchunk:((ci + 1) * t.size) // nchunk]
                for t in tok]
        cap = max(TT, ((max(t.size for t in tokc) + 127) // 128) * 128)
        if cap not in _CACHE:
            _CACHE[cap] = _build(cap)
        nc = _CACHE[cap]
        in_maps = []
        for e in range(E):
            idx = tokc[e]
            n = idx.size
            xe = np.zeros((cap, C), np.float32)
            xe[:n] = xf[idx]
            xs_np = _preround(np.ascontiguousarray(xe.T).reshape(KC, 128, cap))
            in_maps.append({"xs": xs_np, **wmaps[e]})

        trace = os.environ.get("BASS_MOE_TRACE", "0") == "1"
        try:
            res = run_bass_kernel_spmd(nc, in_maps, core_ids=list(range(E)),
                                       trace=trace)
        except ModuleNotFoundError:
            # NTFF profile hook unavailable here; run untraced.
            res = run_bass_kernel_spmd(nc, in_maps, core_ids=list(range(E)))
        LAST_RESULTS = res

        for e in range(E):
            idx = tokc[e]
            n = idx.size
            ye = res.results[e]["yt"].reshape(C, cap).T
            out[idx] += ye[:n]
    return out.reshape(Bb, T, C)



# revision 3
# speedup vs baseline: 1.0121x; 1.0121x over previous
"""Expert-parallel MoE layer for Trainium2 (Bass/Tile, 8 NeuronCores).

Strategy (hardcoded for B=4, T=2048, C=1024, E=8, H=2728, top_k=2):
  - Expert-parallel: core e owns expert e's weights (w1/w2/w3[e]).
  - Host computes the router (top-2 selection AND the softmax combine
    weights g) and performs the all-to-all token dispatch/combine as the
    shard/unshard step.  Selection uses the same stable argsort over fp32
    logits as before (verified to match jax.lax.top_k for this regime).
  - Each core runs the full expert FFN fused in one pass, all in bf16:
      phase A: s = silu(x@w1.T) * (x@w3.T), kept RESIDENT in SBUF (bf16,
               ~94 KiB/partition) -- no DRAM spill/reload of the
               intermediate.
      phase B: y = (s @ w2.T) * g, streamed straight out to HBM.
    bf16 matmuls run at the same PE rate as float32r (1 cycle/row) but
    halve every DMA transfer and SBUF footprint, which is what lets s
    stay resident and the kernel run PE-bound end to end.
"""

import os
import sys
from contextlib import ExitStack

import numpy as np

for _p in ("/opt/trn_rl_repo", "/root/.axon_site/_ro/trn_rl_repo"):
    if os.path.isdir(_p) and _p not in sys.path:
        sys.path.insert(0, _p)

import concourse.mybir as mybir
import concourse.tile as tile
from concourse import bacc
from concourse.bass_utils import run_bass_kernel_spmd

FP32 = mybir.dt.float32
BF16 = mybir.dt.bfloat16
NP_BF16 = mybir.dt.np(mybir.dt.bfloat16)
ALU = mybir.AluOpType
AF = mybir.ActivationFunctionType

E = 8            # experts == cores
C = 1024         # model dim
H0 = 2728        # ffn hidden dim
KC = C // 128    # 8 contraction tiles over C
KH = (H0 + 127) // 128  # 22 tiles over padded H
HP = KH * 128    # 2816
TT = 512         # max token tile (fp32 PSUM bank = 512 floats)
W_LOOK = 4       # w1/w3 h-tile DMA lookahead depth
CAP_MAX = 2176   # per-launch token cap (SBUF budget); split into runs beyond

_CACHE = {}
LAST_RESULTS = None


def _token_tiles(cap):
    """Token tiles: one sub-512 remainder tile FIRST (its smaller x DMA lets
    the first matmul start earlier), then full 512 tiles. bf16 matmuls have
    no narrow-tile penalty."""
    tiles = []
    off = 0
    rem = cap % TT
    if rem:
        tiles.append((0, rem))
        off = rem
    while off < cap:
        tiles.append((off, TT))
        off += TT
    return tiles


def _build(cap):
    """Build + compile the SPMD program for `cap` tokens per core."""
    assert cap % 128 == 0
    tiles = _token_tiles(cap)
    nc = bacc.Bacc("TRN2", target_bir_lowering=False, debug=False, num_devices=E)

    xs = nc.dram_tensor("xs", [KC, 128, cap], BF16, kind="ExternalInput").ap()
    w1s = nc.dram_tensor("w1s", [KH, 128, C], BF16, kind="ExternalInput").ap()
    w3s = nc.dram_tensor("w3s", [KH, 128, C], BF16, kind="ExternalInput").ap()
    w2s = nc.dram_tensor("w2s", [KC, 128, KH * 128], BF16, kind="ExternalInput").ap()
    gs = nc.dram_tensor("gs", [128, cap], BF16, kind="ExternalInput").ap()
    yt = nc.dram_tensor("yt", [KC, 128, cap], BF16, kind="ExternalOutput").ap()

    with tile.TileContext(nc) as tc, ExitStack() as top:
        # resident tensors
        xp = top.enter_context(tc.tile_pool(name="xres", bufs=1))
        x_sb = [xp.tile([128, cap], BF16, tag=f"x{k}", name=f"x_sb{k}")
                for k in range(KC)]
        w2p = top.enter_context(tc.tile_pool(name="w2res", bufs=1))
        w2_sb = [w2p.tile([128, KH, 128], BF16, tag=f"w2_{c}", name=f"w2_sb{c}")
                 for c in range(KC)]
        sres = top.enter_context(tc.tile_pool(name="sres", bufs=1))
        s_sb = sres.tile([128, KH, cap], BF16, name="s_sb")
        gp = top.enter_context(tc.tile_pool(name="gres", bufs=1))
        g_sb = gp.tile([128, cap], BF16, name="g_sb")

        # x streams on the SYNC queue in exact consumption order
        # (t-major, k-minor); per-queue DMA processing is FIFO.
        for to, tw in tiles:
            for k in range(KC):
                nc.sync.dma_start(x_sb[k][:, to:to + tw], xs[k][:, to:to + tw])

        # w1/w3 h-tile stream rides the GPSIMD queue (idle engine, off the
        # startup-critical sync head).
        wst = top.enter_context(tc.tile_pool(name="wst", bufs=W_LOOK))
        w_cur = {}

        def load_wh(h):
            w1_sb = wst.tile([128, C], BF16, tag="w1", name=f"w1_sb{h}")
            nc.gpsimd.dma_start(w1_sb[:], w1s[h])
            w3_sb = wst.tile([128, C], BF16, tag="w3", name=f"w3_sb{h}")
            nc.gpsimd.dma_start(w3_sb[:], w3s[h])
            w_cur[h] = (w1_sb, w3_sb)

        for h in range(min(W_LOOK, KH)):
            load_wh(h)

        # ---- phase A: s = silu(x@w1.T) * (x@w3.T), resident in SBUF ----
        psa = top.enter_context(tc.tile_pool(name="psA", bufs=2, space="PSUM"))
        sap = top.enter_context(tc.tile_pool(name="sap", bufs=3))
        for h in range(KH):
            w1_sb, w3_sb = w_cur.pop(h)
            for to, tw in tiles:
                p1 = psa.tile([128, TT], FP32, tag="p1", name=f"p1_{h}")
                for k in range(KC):
                    nc.tensor.matmul(p1[:, :tw],
                                     w1_sb[:, k * 128:(k + 1) * 128],
                                     x_sb[k][:, to:to + tw],
                                     start=(k == 0), stop=(k == KC - 1))
                p3 = psa.tile([128, TT], FP32, tag="p3", name=f"p3_{h}")
                for k in range(KC):
                    nc.tensor.matmul(p3[:, :tw],
                                     w3_sb[:, k * 128:(k + 1) * 128],
                                     x_sb[k][:, to:to + tw],
                                     start=(k == 0), stop=(k == KC - 1))
                sa = sap.tile([128, TT], FP32, tag="sa", name=f"sa{h}")
                nc.scalar.activation(sa[:, :tw], p1[:, :tw], AF.Silu)
                nc.vector.tensor_tensor(s_sb[:, h, to:to + tw], sa[:, :tw],
                                        p3[:, :tw], op=ALU.mult)
            if h + W_LOOK < KH:
                load_wh(h + W_LOOK)
            # w2 / g loads interleave into the h-loop on the SCALAR queue:
            # issued well after startup, landed long before phase B.
            if 2 <= h < 2 + KC:
                nc.scalar.dma_start(w2_sb[h - 2][:], w2s[h - 2])
            elif h == 2 + KC:
                nc.scalar.dma_start(g_sb[:], gs[:])

        # ---- phase B: y = (s @ w2.T) * g ----
        # Emit full-width tiles first and the remainder tile LAST: the final
        # PSUM->scale->DMA drain is then as short as possible.
        psb = top.enter_context(tc.tile_pool(name="psB", bufs=3, space="PSUM"))
        yp = top.enter_context(tc.tile_pool(name="yst", bufs=4))
        b_order = sorted(range(len(tiles)), key=lambda i: -tiles[i][1])
        for t in b_order:
            to, tw = tiles[t]
            for c in range(KC):
                py = psb.tile([128, TT], FP32, tag="py", name=f"py{t}_{c}")
                for h in range(KH):
                    nc.tensor.matmul(py[:, :tw], w2_sb[c][:, h, :],
                                     s_sb[:, h, to:to + tw],
                                     start=(h == 0), stop=(h == KH - 1))
                yb = yp.tile([128, TT], BF16, tag="y", name=f"yb{t}_{c}")
                nc.vector.tensor_tensor(yb[:, :tw], py[:, :tw],
                                        g_sb[:, to:to + tw], op=ALU.mult)
                nc.sync.dma_start(yt[c, :, to:to + tw], yb[:, :tw])

    nc.compile()
    return nc


def kernel(x, gate_w, w1, w2, w3, top_k):
    global LAST_RESULTS
    x = np.asarray(x, dtype=np.float32)
    gw = np.asarray(gate_w, dtype=np.float32)
    w1 = np.asarray(w1, dtype=np.float32)
    w2 = np.asarray(w2, dtype=np.float32)
    w3 = np.asarray(w3, dtype=np.float32)
    assert int(np.asarray(top_k)) == 2
    Bb, T, Cc = x.shape
    N = Bb * T
    assert Cc == C and w1.shape == (E, H0, C)

    xf = np.ascontiguousarray(x.reshape(N, C))
    # Router on host (dispatch is the sharding step): top-2 selection via
    # stable argsort over fp32 logits (matches jax.lax.top_k here), softmax
    # combine weights in fp32.
    logits = xf @ gw.T
    order = np.argsort(-logits, axis=1, kind="stable")[:, :2]
    vals = np.take_along_axis(logits, order, axis=1)
    ex = np.exp(vals - vals.max(axis=1, keepdims=True))
    gweights = (ex / ex.sum(axis=1, keepdims=True)).astype(np.float32)
    tok, gval = [], []
    for e in range(E):
        sel = order == e                      # [N, 2]
        rows = sel.any(axis=1)
        idx = np.nonzero(rows)[0]
        slot = sel[idx, 1].astype(np.int64)   # 0 if top-1, 1 if top-2
        tok.append(idx)
        gval.append(gweights[idx, slot])

    wmaps = []
    for e in range(E):
        w1t = np.zeros((C, HP), np.float32)
        w1t[:, :H0] = w1[e].T
        w1s_np = np.ascontiguousarray(
            w1t.reshape(KC, 128, KH, 128).transpose(2, 1, 0, 3)
        ).reshape(KH, 128, C).astype(NP_BF16)
        w3t = np.zeros((C, HP), np.float32)
        w3t[:, :H0] = w3[e].T
        w3s_np = np.ascontiguousarray(
            w3t.reshape(KC, 128, KH, 128).transpose(2, 1, 0, 3)
        ).reshape(KH, 128, C).astype(NP_BF16)
        w2t = np.zeros((HP, C), np.float32)
        w2t[:H0] = w2[e].T
        w2s_np = np.ascontiguousarray(
            w2t.reshape(KH, 128, KC, 128).transpose(2, 1, 0, 3)
        ).reshape(KC, 128, KH * 128).astype(NP_BF16)
        wmaps.append({"w1s": w1s_np, "w3s": w3s_np, "w2s": w2s_np})

    out = np.zeros((N, C), np.float32)
    # normally one launch; if an expert ever holds > CAP_MAX tokens, split
    # tokens into several SPMD launches (FFN + gate weight are per-token)
    nchunk = (max(t.size for t in tok) + CAP_MAX - 1) // CAP_MAX
    for ci in range(nchunk):
        tokc = [t[(ci * t.size) // nchunk:((ci + 1) * t.size) // nchunk]
                for t in tok]
        gvalc = [g[(ci * g.size) // nchunk:((ci + 1) * g.size) // nchunk]
                 for g in gval]
        cap = max(TT, ((max(t.size for t in tokc) + 127) // 128) * 128)
        if cap not in _CACHE:
            _CACHE[cap] = _build(cap)
        nc = _CACHE[cap]
        in_maps = []
        for e in range(E):
            idx = tokc[e]
            n = idx.size
            xe = np.zeros((cap, C), np.float32)
            xe[:n] = xf[idx]
            xs_np = np.ascontiguousarray(xe.T).reshape(KC, 128, cap).astype(NP_BF16)
            ge = np.zeros((cap,), np.float32)
            ge[:n] = gvalc[e]
            gs_np = np.broadcast_to(ge.astype(NP_BF16), (128, cap)).copy()
            in_maps.append({"xs": xs_np, "gs": gs_np, **wmaps[e]})

        trace = os.environ.get("BASS_MOE_TRACE", "0") == "1"
        try:
            res = run_bass_kernel_spmd(nc, in_maps, core_ids=list(range(E)),
                                       trace=trace)
        except ModuleNotFoundError:
            # NTFF profile hook unavailable here; run untraced.
            res = run_bass_kernel_spmd(nc, in_maps, core_ids=list(range(E)))
        LAST_RESULTS = res

        for e in range(E):
            idx = tokc[e]
            n = idx.size
            ye = res.results[e]["yt"].astype(np.float32).reshape(C, cap).T
            out[idx] += ye[:n]
    return out.reshape(Bb, T, C)


# revision 34
# speedup vs baseline: 1.0531x; 1.0405x over previous
"""Expert-parallel MoE layer for Trainium2 (Bass/Tile, 8 NeuronCores).

Strategy (hardcoded for B=4, T=2048, C=1024, E=8, H=2728, top_k=2):
  - Expert-parallel: core e owns expert e's weights (w1/w2/w3[e]).
  - Host computes the router (top-2 selection AND the softmax combine
    weights g) and performs the all-to-all token dispatch/combine as the
    shard/unshard step.  Selection uses the same stable argsort over fp32
    logits as before (verified to match jax.lax.top_k for this regime).
  - Each core runs the full expert FFN fused in one pass, all in bf16:
      phase A: s = silu(x@w1.T) * (x@w3.T), kept RESIDENT in SBUF (bf16,
               ~94 KiB/partition) -- no DRAM spill/reload of the
               intermediate.
      phase B: y = (s @ w2.T) * g, streamed straight out to HBM.
    bf16 matmuls run at the same PE rate as float32r (1 cycle/row) but
    halve every DMA transfer and SBUF footprint, which is what lets s
    stay resident and the kernel run PE-bound end to end.
"""

import os
import sys
from contextlib import ExitStack

import numpy as np

for _p in ("/opt/trn_rl_repo", "/root/.axon_site/_ro/trn_rl_repo"):
    if os.path.isdir(_p) and _p not in sys.path:
        sys.path.insert(0, _p)

import concourse.mybir as mybir
import concourse.tile as tile
from concourse.tile_rust import add_dep_helper
from concourse import bacc
from concourse.bass_utils import run_bass_kernel_spmd

FP32 = mybir.dt.float32
BF16 = mybir.dt.bfloat16
NP_BF16 = mybir.dt.np(mybir.dt.bfloat16)
ALU = mybir.AluOpType
AF = mybir.ActivationFunctionType

E = 8            # experts == cores
C = 1024         # model dim
H0 = 2728        # ffn hidden dim
KC = C // 128    # 8 contraction tiles over C
KH = (H0 + 127) // 128  # 22 tiles over padded H
HP = KH * 128    # 2816
TT = 512         # max token tile (fp32 PSUM bank = 512 floats)
W_LOOK = 5       # w1/w3 h-tile buffer depth (slack decouples slot waits)
H_PRE = 4        # leading h-tiles interleaved token-major (paces the x stream)
N_WARM = 48      # PE warmup matmuls covering the p-state ramp at startup
CAP_MAX = 2176   # per-launch token cap (SBUF budget); split into runs beyond

_CACHE = {}
LAST_RESULTS = None


def _token_tiles(cap):
    """Token tiles: a 256 tile first (small x DMA -> earliest first matmul),
    full 512 tiles in the middle, and the remainder folded into a >=256
    trailing tile. All tiles are >=256 tokens so every x/s/y DMA moves
    >=512B contiguous runs (below that the DMA model pays a 2x penalty).
    bf16 matmuls have no narrow-tile penalty. Phase B iterates smallest
    LAST, keeping the final drain short."""
    if cap <= TT:
        widths = [cap]
    else:
        widths = [384]
        r = cap - 384
        while r > 896:
            widths.append(TT)
            r -= TT
        widths += {128: [128], 256: [256], 384: [384], 512: [256, 256],
                   640: [384, 256], 768: [512, 256], 896: [512, 384]}[r]
    assert sum(widths) == cap
    tiles = []
    off = 0
    for w in widths:
        tiles.append((off, w))
        off += w
    return tiles


def _build(cap):
    """Build + compile the SPMD program for `cap` tokens per core."""
    assert cap % 128 == 0
    tiles = _token_tiles(cap)
    nc = bacc.Bacc("TRN2", target_bir_lowering=False, debug=False, num_devices=E)

    xs = nc.dram_tensor("xs", [KC, 128, cap], BF16, kind="ExternalInput").ap()
    w1s = nc.dram_tensor("w1s", [KH, 128, C], BF16, kind="ExternalInput").ap()
    w3s = nc.dram_tensor("w3s", [KH, 128, C], BF16, kind="ExternalInput").ap()
    w2s = nc.dram_tensor("w2s", [KC, 128, KH * 128], BF16, kind="ExternalInput").ap()
    gs = nc.dram_tensor("gs", [128, cap], BF16, kind="ExternalInput").ap()
    yt = nc.dram_tensor("yt", [KC, 128, cap], BF16, kind="ExternalOutput").ap()

    with tile.TileContext(nc) as tc, ExitStack() as top:
        # resident tensors
        xp = top.enter_context(tc.tile_pool(name="xres", bufs=1))
        x_sb = xp.tile([128, KC, cap], BF16, name="x_sb")
        w2p = top.enter_context(tc.tile_pool(name="w2res", bufs=1))
        w2_sb = [w2p.tile([128, KH, 128], BF16, tag=f"w2_{c}", name=f"w2_sb{c}")
                 for c in range(KC)]
        sres = top.enter_context(tc.tile_pool(name="sres", bufs=1))
        s_sb = sres.tile([128, KH, cap], BF16, name="s_sb")
        gp = top.enter_context(tc.tile_pool(name="gres", bufs=1))
        g_sb = gp.tile([128, cap], BF16, name="g_sb")

        # Startup: DMA issue costs ~0.65us of SEQ time per dma_start and the
        # DMA device is serial, so the startup-critical stream rides the
        # SYNC queue as few, whole-tile DMAs in exact consumption order:
        # w1[h0], x t0, w3[h0], the h1..h3 pairs, then the remaining x
        # tiles. Later weight pairs are dependency-delayed onto the GPSIMD
        # queue so they cannot cut in front of this stream on the shared
        # DMA device.
        wst = top.enter_context(tc.tile_pool(name="wst", bufs=W_LOOK))
        w_cur = {}

        def load_wh(h, eng=None, dep=None):
            w1_sb = wst.tile([128, C], BF16, tag="w1", name=f"w1_sb{h}")
            w3_sb = wst.tile([128, C], BF16, tag="w3", name=f"w3_sb{h}")
            d1 = (eng or nc.gpsimd).dma_start(w1_sb[:], w1s[h])
            d3 = (eng or nc.gpsimd).dma_start(w3_sb[:], w3s[h])
            if dep is not None:
                add_dep_helper(d1.ins, dep.ins, reason="stagger w stream")
                add_dep_helper(d3.ins, dep.ins, reason="stagger w stream")
            w_cur[h] = (w1_sb, w3_sb)

        xin = xs.rearrange("k p t -> p k t")
        w1_sb0 = wst.tile([128, C], BF16, tag="w1", name="w1_sb0")
        w3_sb0 = wst.tile([128, C], BF16, tag="w3", name="w3_sb0")
        w_cur[0] = (w1_sb0, w3_sb0)
        nc.sync.dma_start(w1_sb0[:], w1s[0])
        to0, tw0 = tiles[0]
        nc.sync.dma_start(x_sb[:, :, to0:to0 + tw0], xin[:, :, to0:to0 + tw0])
        nc.sync.dma_start(w3_sb0[:], w3s[0])
        for h in range(1, min(H_PRE, KH)):
            load_wh(h, eng=nc.sync)
        for to, tw in tiles[1:]:
            nc.sync.dma_start(x_sb[:, :, to:to + tw], xin[:, :, to:to + tw])

        # PE warmup: dependency-free matmuls on a memset tile keep the PE
        # busy through its p-state ramp while the first real DMAs land, so
        # real matmuls start at full clock. Sized to end just as the first
        # weight/x chunks arrive.
        wup = top.enter_context(tc.tile_pool(name="wup", bufs=1))
        wu = wup.tile([128, 128], BF16, name="wu")
        wups = top.enter_context(tc.tile_pool(name="wups", bufs=1, space="PSUM"))
        wu_ps = wups.tile([128, 128], FP32, name="wu_ps")
        nc.vector.memset(wu[:], 0.0)
        for _ in range(N_WARM):
            nc.tensor.matmul(wu_ps[:], wu[:], wu[:], start=True, stop=True)

        # ---- phase A: s = silu(x@w1.T) * (x@w3.T), resident in SBUF ----
        psa = top.enter_context(tc.tile_pool(name="psA", bufs=2, space="PSUM"))
        sap = top.enter_context(tc.tile_pool(name="sap", bufs=3))
        anchors = {}

        anchors0 = {}

        def emit_ht(h, to, tw):
            w1_sb, w3_sb = w_cur[h]
            p1 = psa.tile([128, TT], FP32, tag="p1", name=f"p1_{h}")
            for k in range(KC):
                mm = nc.tensor.matmul(p1[:, :tw],
                                      w1_sb[:, k * 128:(k + 1) * 128],
                                      x_sb[:, k, to:to + tw],
                                      start=(k == 0), stop=(k == KC - 1))
                if k == 0:
                    anchors0.setdefault(h, mm)
            p3 = psa.tile([128, TT], FP32, tag="p3", name=f"p3_{h}")
            for k in range(KC):
                mm = nc.tensor.matmul(p3[:, :tw],
                                      w3_sb[:, k * 128:(k + 1) * 128],
                                      x_sb[:, k, to:to + tw],
                                      start=(k == 0), stop=(k == KC - 1))
            anchors[h] = mm
            sa = sap.tile([128, TT], FP32, tag="sa", name=f"sa{h}")
            nc.scalar.activation(sa[:, :tw], p1[:, :tw], AF.Silu)
            nc.vector.tensor_tensor(s_sb[:, h, to:to + tw], sa[:, :tw],
                                    p3[:, :tw], op=ALU.mult)

        # The first H_PRE h-tiles run token-major so the PE's x consumption
        # paces the incoming x stream (x tile t is only needed after H_PRE
        # passes over tiles < t) -- no PE stall while x streams in.
        for ti, (to, tw) in enumerate(tiles):
            for h in range(min(H_PRE, KH)):
                emit_ht(h, to, tw)
            if ti == 0 and H_PRE < KH:
                load_wh(H_PRE, dep=anchors[0])
        if H_PRE + 1 < KH:
            load_wh(H_PRE + 1)
        for h in range(H_PRE, KH):
            for to, tw in tiles:
                emit_ht(h, to, tw)
            if h + 2 < KH:
                load_wh(h + 2)
            # w2 / g loads ride the SCALAR queue (separate from the w1/w3
            # stream), dependency-anchored to the current h-tile's last
            # matmul so the tile scheduler cannot hoist them into the
            # startup x window; spread every other h iteration so they
            # never head-block the queue or crowd the DMA device.
            if h % 2 == 0 and H_PRE <= h <= H_PRE + 2 * (KC - 1):
                c = (h - H_PRE) // 2
                w2dma = nc.scalar.dma_start(w2_sb[c][:], w2s[c])
                add_dep_helper(w2dma.ins, anchors[h].ins,
                               reason="delay w2 prefetch")
            elif h == H_PRE + 2 * KC - 1:
                gdma = nc.scalar.dma_start(g_sb[:], gs[:])
                add_dep_helper(gdma.ins, anchors[h].ins,
                               reason="delay g load")

        # ---- phase B: y = (s @ w2.T) * g ----
        # Emit full-width tiles first and the remainder tile LAST: the final
        # PSUM->scale->DMA drain is then as short as possible.
        psb = top.enter_context(tc.tile_pool(name="psB", bufs=3, space="PSUM"))
        yp = top.enter_context(tc.tile_pool(name="yst", bufs=4))
        b_order = sorted(range(len(tiles)), key=lambda i: -tiles[i][1])
        for t in b_order:
            to, tw = tiles[t]
            for c in range(KC):
                py = psb.tile([128, TT], FP32, tag="py", name=f"py{t}_{c}")
                for h in range(KH):
                    nc.tensor.matmul(py[:, :tw], w2_sb[c][:, h, :],
                                     s_sb[:, h, to:to + tw],
                                     start=(h == 0), stop=(h == KH - 1))
                yb = yp.tile([128, TT], BF16, tag="y", name=f"yb{t}_{c}")
                nc.vector.tensor_tensor(yb[:, :tw], py[:, :tw],
                                        g_sb[:, to:to + tw], op=ALU.mult)
                nc.sync.dma_start(yt[c, :, to:to + tw], yb[:, :tw])

    nc.compile()
    return nc


def kernel(x, gate_w, w1, w2, w3, top_k):
    global LAST_RESULTS
    x = np.asarray(x, dtype=np.float32)
    gw = np.asarray(gate_w, dtype=np.float32)
    w1 = np.asarray(w1, dtype=np.float32)
    w2 = np.asarray(w2, dtype=np.float32)
    w3 = np.asarray(w3, dtype=np.float32)
    assert int(np.asarray(top_k)) == 2
    Bb, T, Cc = x.shape
    N = Bb * T
    assert Cc == C and w1.shape == (E, H0, C)

    xf = np.ascontiguousarray(x.reshape(N, C))
    # Router on host (dispatch is the sharding step): top-2 selection via
    # stable argsort over fp32 logits (matches jax.lax.top_k here), softmax
    # combine weights in fp32.
    logits = xf @ gw.T
    order = np.argsort(-logits, axis=1, kind="stable")[:, :2]
    vals = np.take_along_axis(logits, order, axis=1)
    ex = np.exp(vals - vals.max(axis=1, keepdims=True))
    gweights = (ex / ex.sum(axis=1, keepdims=True)).astype(np.float32)
    tok, gval = [], []
    for e in range(E):
        sel = order == e                      # [N, 2]
        rows = sel.any(axis=1)
        idx = np.nonzero(rows)[0]
        slot = sel[idx, 1].astype(np.int64)   # 0 if top-1, 1 if top-2
        tok.append(idx)
        gval.append(gweights[idx, slot])

    wmaps = []
    for e in range(E):
        w1t = np.zeros((C, HP), np.float32)
        w1t[:, :H0] = w1[e].T
        w1s_np = np.ascontiguousarray(
            w1t.reshape(KC, 128, KH, 128).transpose(2, 1, 0, 3)
        ).reshape(KH, 128, C).astype(NP_BF16)
        w3t = np.zeros((C, HP), np.float32)
        w3t[:, :H0] = w3[e].T
        w3s_np = np.ascontiguousarray(
            w3t.reshape(KC, 128, KH, 128).transpose(2, 1, 0, 3)
        ).reshape(KH, 128, C).astype(NP_BF16)
        w2t = np.zeros((HP, C), np.float32)
        w2t[:H0] = w2[e].T
        w2s_np = np.ascontiguousarray(
            w2t.reshape(KH, 128, KC, 128).transpose(2, 1, 0, 3)
        ).reshape(KC, 128, KH * 128).astype(NP_BF16)
        wmaps.append({"w1s": w1s_np, "w3s": w3s_np, "w2s": w2s_np})

    out = np.zeros((N, C), np.float32)
    # normally one launch; if an expert ever holds > CAP_MAX tokens, split
    # tokens into several SPMD launches (FFN + gate weight are per-token)
    nchunk = (max(t.size for t in tok) + CAP_MAX - 1) // CAP_MAX
    for ci in range(nchunk):
        tokc = [t[(ci * t.size) // nchunk:((ci + 1) * t.size) // nchunk]
                for t in tok]
        gvalc = [g[(ci * g.size) // nchunk:((ci + 1) * g.size) // nchunk]
                 for g in gval]
        cap = max(TT, ((max(t.size for t in tokc) + 127) // 128) * 128)
        if cap not in _CACHE:
            _CACHE[cap] = _build(cap)
        nc = _CACHE[cap]
        in_maps = []
        for e in range(E):
            idx = tokc[e]
            n = idx.size
            xe = np.zeros((cap, C), np.float32)
            xe[:n] = xf[idx]
            xs_np = np.ascontiguousarray(xe.T).reshape(KC, 128, cap).astype(NP_BF16)
            ge = np.zeros((cap,), np.float32)
            ge[:n] = gvalc[e]
            gs_np = np.broadcast_to(ge.astype(NP_BF16), (128, cap)).copy()
            in_maps.append({"xs": xs_np, "gs": gs_np, **wmaps[e]})

        trace = os.environ.get("BASS_MOE_TRACE", "0") == "1"
        try:
            res = run_bass_kernel_spmd(nc, in_maps, core_ids=list(range(E)),
                                       trace=trace)
        except ModuleNotFoundError:
            # NTFF profile hook unavailable here; run untraced.
            res = run_bass_kernel_spmd(nc, in_maps, core_ids=list(range(E)))
        LAST_RESULTS = res

        for e in range(E):
            idx = tokc[e]
            n = idx.size
            ye = res.results[e]["yt"].astype(np.float32).reshape(C, cap).T
            out[idx] += ye[:n]
    return out.reshape(Bb, T, C)
